# revision 6
# baseline (speedup 1.0000x reference)
"""DeepTensorNeuralNetwork (DTNN / gnn_message_passing) Trainium2 kernel.

Math (per reference):
    d_sum = distance.sum(axis=2)                                  # (B,N,R)
    for l in 0..2:
        cf = x @ Wcf[l].T + bcf[l]                                # (B,N,H)
        df = d_sum @ Wdf[l].T + N*bdf[l]                          # (B,N,H)
        h  = (cf*df) @ Wfc[l].T                                   # (B,N,F)
        x  = h + tanh(h)
    g = x.sum(axis=1); out = (g @ fc0.T + b0) @ ow.T + ob         # (B,1)

Strategy: data-parallel over batch across 8 NeuronCores (8 batches each).
The dominant cost is streaming `distance` (33.5 MB/core) from HBM at
~358 GB/s (~94us floor). The j-reduction is a binary fold tree:
GpSimd does the first fold (8192->4096 elems/lane, ~8.9us/batch,
otherwise-idle engine), DVE does the rest (4096->64, ~5us/batch) --
both hide under the ~11.7us/batch DMA. (DVE tensor_reduce measured
1.64 cyc/elem = too slow; folds run at 1 cyc/output.) Layer matmuls
process batch PAIRS (moving dim N=256) on the PE in fp32; ACT does
PSUM->SBUF bias copies and tanh; DVE does cf*df and the residual add.
The affine head (fc0 + out) is folded on the host into a single
length-F vector + scalar bias (mathematically identical). All constants
ship in ONE packed array -> one DMA -> one wait sem.
"""

import numpy as np

B, N, F, R, H = 64, 128, 128, 64, 256
L = 3
NCORES = 8
BL = B // NCORES  # batches per core

# wpack column layout (128 partitions):
#   [0, 768)       wcf lhsT   : wpack[f, l*H + h]          = Wcf_w[l, h, f]
#   [768, 1536)    wfc lhsT   : wpack[hc, 768+(l*2+c)*F+f] = Wfc_w[l, f, c*128+hc]
#   [1536, 1542)   cf bias    : wpack[h, 1536 + l*2+c]     = Wcf_b[l, c*128+h]
#   [1542, 1548)   df bias    : wpack[h, 1542 + l*2+c]     = N * Wdf_b[l, c*128+h]
#   [1548]         head lhsT  : wpack[f, 1548]             = (out_w @ fc0_w)[0, f]
#   [1552, 2320)   wdf lhsT   : wpack[r, 1552 + l*H + h]   = Wdf_w[l, h, r]   (rows 0:64)
#   [2320, 3344)   x          : wpack[f, 2320 + b*N + n]   = x[b_local, n, f]
#   [3344, 3472)   identity 128x128
XOFF = 2320
IDOFF = 3344
WCOLS = 3472

_CACHE = {}


def _build_program():
    import concourse.bass as bass
    from concourse import bacc
    import concourse.tile as tile
    from concourse import mybir

    f32 = mybir.dt.float32
    AX = mybir.AxisListType
    AF = mybir.ActivationFunctionType

    nc = bacc.Bacc("TRN2")
    dist = nc.declare_dram_parameter("dist", [BL, N, N, R], f32, isOutput=False)
    wpack = nc.declare_dram_parameter("wpack", [128, WCOLS], f32, isOutput=False)
    out_ext = nc.declare_dram_parameter("out", [BL, 1], f32, isOutput=True)

    with tile.TileContext(nc) as tc:
        with (
            tc.tile_pool(name="consts", bufs=1) as consts,
            tc.tile_pool(name="dist", bufs=3) as dist_pool,
            tc.tile_pool(name="fold", bufs=2) as fold_pool,
            tc.tile_pool(name="dsum", bufs=2) as dsum_pool,
            tc.tile_pool(name="work", bufs=3) as work,
            tc.tile_pool(name="ps1", bufs=1, space="PSUM") as ps1,
            tc.tile_pool(name="ps2", bufs=2, space="PSUM") as ps2,
        ):
            wp = consts.tile([128, WCOLS], f32)
            nc.sync.dma_start(out=wp, in_=wpack[:, :])
            ident = wp[:, IDOFF : IDOFF + 128]
            out_acc = consts.tile([1, BL], f32)

            def wcf_l(l, c):
                return wp[:, l * H + c * 128 : l * H + (c + 1) * 128]

            def wdf_l(l, c):
                o = 1552 + l * H + c * 128
                return wp[0:R, o : o + 128]

            def wfc_l(l, c):
                o = 768 + (l * 2 + c) * F
                return wp[:, o : o + F]

            def bcf_l(l, c):
                o = 1536 + l * 2 + c
                return wp[:, o : o + 1]

            def bdf_l(l, c):
                o = 1542 + l * 2 + c
                return wp[:, o : o + 1]

            def reduce_j(b):
                """Load dist[b], fold j 128->1: GpSimd first fold, DVE rest.
                Returns dsum tile (N, R)."""
                dist_t = dist_pool.tile([N, N * R], f32, tag="dist")
                nc.sync.dma_start(out=dist_t, in_=dist[b, :, :, :].rearrange("n j r -> n (j r)"))
                fA = fold_pool.tile([N, 4096], f32, tag="fA")
                nc.gpsimd.tensor_add(fA, dist_t[:, 0:4096], dist_t[:, 4096:8192])
                fB = fold_pool.tile([N, 2048], f32, tag="fB")
                nc.vector.tensor_add(fB, fA[:, 0:2048], fA[:, 2048:4096])
                fC = fold_pool.tile([N, 1024], f32, tag="fC")
                nc.vector.tensor_add(fC, fB[:, 0:1024], fB[:, 1024:2048])
                fD = fold_pool.tile([N, 512], f32, tag="fD")
                nc.vector.tensor_add(fD, fC[:, 0:512], fC[:, 512:1024])
                fE = fold_pool.tile([N, 256], f32, tag="fE")
                nc.vector.tensor_add(fE, fD[:, 0:256], fD[:, 256:512])
                fF = fold_pool.tile([N, 128], f32, tag="fF")
                nc.vector.tensor_add(fF, fE[:, 0:128], fE[:, 128:256])
                dsum = dsum_pool.tile([N, R], f32, tag="dsum")
                nc.vector.tensor_add(dsum, fF[:, 0:64], fF[:, 64:128])
                return dsum

            for p in range(BL // 2):
                b0, b1 = 2 * p, 2 * p + 1
                # d_sum for both batches; transpose to (r, n) and pack pair
                dsT = dsum_pool.tile([R, 2 * N], f32, tag="dsT")
                for k, b in enumerate((b0, b1)):
                    dsum = reduce_j(b)
                    trp = ps1.tile([R, N], f32, tag="tr")
                    nc.tensor.transpose(trp, dsum, ident)
                    nc.scalar.activation(
                        out=dsT[:, k * N : (k + 1) * N], in_=trp, func=AF.Copy
                    )

                xc = wp[:, XOFF + b0 * N : XOFF + (b1 + 1) * N]  # (F, 2N)
                for l in range(L):
                    ms = []
                    for c in range(2):
                        cfp = ps1.tile([128, 2 * N], f32, tag=f"cf{c}")
                        nc.tensor.matmul(cfp, wcf_l(l, c), xc, start=True, stop=True)
                        dfp = ps1.tile([128, 2 * N], f32, tag=f"df{c}")
                        nc.tensor.matmul(dfp, wdf_l(l, c), dsT, start=True, stop=True)
                        cfs = work.tile([128, 2 * N], f32, tag=f"cfs{c}")
                        nc.scalar.activation(out=cfs, in_=cfp, func=AF.Identity, bias=bcf_l(l, c))
                        dfs = work.tile([128, 2 * N], f32, tag=f"dfs{c}")
                        nc.scalar.activation(out=dfs, in_=dfp, func=AF.Identity, bias=bdf_l(l, c))
                        m = work.tile([128, 2 * N], f32, tag=f"m{c}")
                        nc.vector.tensor_mul(m, cfs, dfs)
                        ms.append(m)
                    hp = ps2.tile([F, 2 * N], f32, tag="h")
                    nc.tensor.matmul(hp, wfc_l(l, 0), ms[0], start=True, stop=False)
                    nc.tensor.matmul(hp, wfc_l(l, 1), ms[1], start=False, stop=True)
                    th = work.tile([F, 2 * N], f32, tag="t")
                    nc.scalar.activation(out=th, in_=hp, func=AF.Tanh)
                    xn = work.tile([F, 2 * N], f32, tag="x")
                    nc.vector.tensor_add(xn, hp, th)
                    xc = xn

                # head: out[b] = sum_n sum_f x[f, n] * w_head[f]
                hd = ps1.tile([1, 2 * N], f32, tag="hd")
                nc.tensor.matmul(hd, wp[:, 1548:1549], xc, start=True, stop=True)
                nc.vector.tensor_reduce(
                    out=out_acc[0:1, b0 : b0 + 2],
                    in_=hd.rearrange("o (b n) -> o b n", b=2),
                    axis=AX.X,
                    op=mybir.AluOpType.add,
                )

            nc.sync.dma_start(out=out_ext.rearrange("b o -> o b"), in_=out_acc)

    return nc


def _host_pack(x, Wcf_w, Wcf_b, Wdf_w, Wdf_b, Wfc_w, fc0_w, fc0_b, out_w, out_b):
    f = np.float32
    base = np.zeros((128, WCOLS), f)
    base[:, 0:768] = np.asarray(Wcf_w, f).transpose(2, 0, 1).reshape(128, L * H)
    base[:, 768:1536] = (
        np.asarray(Wfc_w, f).reshape(L, F, 2, 128).transpose(3, 0, 2, 1).reshape(128, L * 2 * F)
    )
    base[:, 1536:1542] = np.asarray(Wcf_b, f).reshape(L, 2, 128).transpose(2, 0, 1).reshape(128, 6)
    base[:, 1542:1548] = (
        (N * np.asarray(Wdf_b, f)).reshape(L, 2, 128).transpose(2, 0, 1).reshape(128, 6)
    )
    w_head = (np.asarray(out_w, np.float64) @ np.asarray(fc0_w, np.float64))[0]  # (F,)
    base[:, 1548] = w_head.astype(f)
    base[0:R, 1552:2320] = np.asarray(Wdf_w, f).transpose(2, 0, 1).reshape(R, L * H)
    base[:, IDOFF : IDOFF + 128] = np.eye(128, dtype=f)

    b_head = float((np.asarray(out_w, np.float64) @ np.asarray(fc0_b, np.float64)
                    + np.asarray(out_b, np.float64)).reshape(()))

    x_t = np.asarray(x, f).transpose(0, 2, 1)  # (B, F, N)
    wpacks = []
    for i in range(NCORES):
        wp = base.copy()
        # wp[f, XOFF + b*N + n] = x_t[global_b, f, n]
        wp[:, XOFF : XOFF + BL * N] = (
            x_t[i * BL : (i + 1) * BL].transpose(1, 0, 2).reshape(128, BL * N)
        )
        wpacks.append(wp)
    return wpacks, b_head


def run(trace=False, **inputs):
    from concourse.bass_utils import run_bass_kernel_spmd

    distance = np.ascontiguousarray(np.asarray(inputs["distance"], np.float32))
    wpacks, b_head = _host_pack(
        inputs["x"], inputs["Wcf_w"], inputs["Wcf_b"], inputs["Wdf_w"], inputs["Wdf_b"],
        inputs["Wfc_w"], inputs["fc0_w"], inputs["fc0_b"], inputs["out_w"], inputs["out_b"],
    )

    if "nc" not in _CACHE:
        nc = _build_program()
        nc.finalize()
        _CACHE["nc"] = nc
    nc = _CACHE["nc"]

    in_maps = []
    for i in range(NCORES):
        in_maps.append({
            "dist": np.ascontiguousarray(distance[i * BL : (i + 1) * BL]),
            "wpack": wpacks[i],
        })
    res = run_bass_kernel_spmd(nc, in_maps, list(range(NCORES)), trace=trace)
    out = np.concatenate([res.results[i]["out"] for i in range(NCORES)], axis=0)
    out = (out.astype(np.float64) + b_head).astype(np.float32)
    return out, res


def kernel(**inputs):
    out, _ = run(trace=False, **inputs)
    return out


# revision 8
# speedup vs baseline: 1.2825x; 1.2825x over previous
"""DeepTensorNeuralNetwork (DTNN / gnn_message_passing) Trainium2 kernel.

Math (per reference):
    d_sum = distance.sum(axis=2)                                  # (B,N,R)
    for l in 0..2:
        cf = x @ Wcf[l].T + bcf[l]                                # (B,N,H)
        df = d_sum @ Wdf[l].T + N*bdf[l]                          # (B,N,H)
        h  = (cf*df) @ Wfc[l].T                                   # (B,N,F)
        x  = h + tanh(h)
    g = x.sum(axis=1); out = (g @ fc0.T + b0) @ ow.T + ob         # (B,1)

Strategy: data-parallel over batch across 8 NeuronCores (8 batches each).
The dominant cost is streaming `distance` (33.5 MB/core) from HBM at
~358 GB/s (~94us floor). The j-reduction is a binary fold tree:
GpSimd does the first fold (8192->4096 elems/lane, ~8.9us/batch,
otherwise-idle engine), DVE does the rest (4096->64, ~5us/batch) --
both hide under the ~11.7us/batch DMA. (DVE tensor_reduce measured
1.64 cyc/elem = too slow; folds run at 1 cyc/output.) Layer matmuls
process batch PAIRS (moving dim N=256) on the PE in fp32; ACT does
PSUM->SBUF bias copies and tanh; DVE does cf*df and the residual add.
The affine head (fc0 + out) is folded on the host into a single
length-F vector + scalar bias (mathematically identical). All constants
ship in ONE packed array -> one DMA -> one wait sem.
"""

import numpy as np

B, N, F, R, H = 64, 128, 128, 64, 256
L = 3
NCORES = 8
BL = B // NCORES  # batches per core

# wpack column layout (128 partitions):
#   [0, 768)       wcf lhsT   : wpack[f, l*H + h]          = Wcf_w[l, h, f]
#   [768, 1536)    wfc lhsT   : wpack[hc, 768+(l*2+c)*F+f] = Wfc_w[l, f, c*128+hc]
#   [1536, 1542)   cf bias    : wpack[h, 1536 + l*2+c]     = Wcf_b[l, c*128+h]
#   [1542, 1548)   df bias    : wpack[h, 1542 + l*2+c]     = N * Wdf_b[l, c*128+h]
#   [1548]         head lhsT  : wpack[f, 1548]             = (out_w @ fc0_w)[0, f]
#   [1552, 2320)   wdf lhsT   : wpack[r, 1552 + l*H + h]   = Wdf_w[l, h, r]   (rows 0:64)
#   [2320, 3344)   x          : wpack[f, 2320 + b*N + n]   = x[b_local, n, f]
#   [3344, 3472)   identity 128x128
XOFF = 2320
IDOFF = 3344
WCOLS = 3472

_CACHE = {}


def _build_program():
    import concourse.bass as bass
    from concourse import bacc
    import concourse.tile as tile
    from concourse import mybir

    f32 = mybir.dt.float32
    AX = mybir.AxisListType
    AF = mybir.ActivationFunctionType

    nc = bacc.Bacc("TRN2")
    dist = nc.declare_dram_parameter("dist", [BL, N, N, R], f32, isOutput=False)
    wpack = nc.declare_dram_parameter("wpack", [128, WCOLS], f32, isOutput=False)
    out_ext = nc.declare_dram_parameter("out", [BL, 1], f32, isOutput=True)

    with tile.TileContext(nc) as tc:
        with (
            tc.tile_pool(name="consts", bufs=1) as consts,
            tc.tile_pool(name="dist", bufs=3) as dist_pool,
            tc.tile_pool(name="fold", bufs=2) as fold_pool,
            tc.tile_pool(name="dsum", bufs=2) as dsum_pool,
            tc.tile_pool(name="work", bufs=3) as work,
            tc.tile_pool(name="ps1", bufs=1, space="PSUM") as ps1,
            tc.tile_pool(name="ps2", bufs=2, space="PSUM") as ps2,
        ):
            # issue the first distance loads BEFORE the weight pack so the
            # fold pipeline starts as early as possible (Sync queue is FIFO)
            dist_tiles = {}
            for b in range(2):
                dist_t = dist_pool.tile([N, N * R], f32, tag="dist")
                dflat = dist[b, :, :, :].rearrange("n j r -> n (j r)")
                nc.sync.dma_start(out=dist_t[:, 0:4096], in_=dflat[:, 0:4096])
                nc.sync.dma_start(out=dist_t[:, 4096:8192], in_=dflat[:, 4096:8192])
                dist_tiles[b] = dist_t

            wp = consts.tile([128, WCOLS], f32)
            nc.sync.dma_start(out=wp, in_=wpack[:, :])
            ident = wp[:, IDOFF : IDOFF + 128]
            out_acc = consts.tile([1, BL], f32)

            def wcf_l(l, c):
                return wp[:, l * H + c * 128 : l * H + (c + 1) * 128]

            def wdf_l(l, c):
                o = 1552 + l * H + c * 128
                return wp[0:R, o : o + 128]

            def wfc_l(l, c):
                o = 768 + (l * 2 + c) * F
                return wp[:, o : o + F]

            def bcf_l(l, c):
                o = 1536 + l * 2 + c
                return wp[:, o : o + 1]

            def bdf_l(l, c):
                o = 1542 + l * 2 + c
                return wp[:, o : o + 1]

            def half_fold(src, half, tag_suffix):
                """DVE fold tree over one j-half (4096 elems/lane -> 64)."""
                o = half * 4096
                fB = fold_pool.tile([N, 2048], f32, tag=f"fB{tag_suffix}")
                nc.vector.tensor_add(fB, src[:, o : o + 2048], src[:, o + 2048 : o + 4096])
                fC = fold_pool.tile([N, 1024], f32, tag=f"fC{tag_suffix}")
                nc.vector.tensor_add(fC, fB[:, 0:1024], fB[:, 1024:2048])
                fD = fold_pool.tile([N, 512], f32, tag=f"fD{tag_suffix}")
                nc.vector.tensor_add(fD, fC[:, 0:512], fC[:, 512:1024])
                fE = fold_pool.tile([N, 256], f32, tag=f"fE{tag_suffix}")
                nc.vector.tensor_add(fE, fD[:, 0:256], fD[:, 256:512])
                fF = fold_pool.tile([N, 128], f32, tag=f"fF{tag_suffix}")
                nc.vector.tensor_add(fF, fE[:, 0:128], fE[:, 128:256])
                fG = fold_pool.tile([N, 64], f32, tag=f"fG{tag_suffix}")
                nc.vector.tensor_add(fG, fF[:, 0:64], fF[:, 64:128])
                return fG

            def reduce_j(b):
                """Fold j 128->1 on DVE (two independent half-trees, earlier
                start on the first DMA half). Returns dsum tile (N, R)."""
                dist_t = dist_tiles.pop(b)
                if b + 2 < BL:  # prefetch two batches ahead
                    nb = b + 2
                    nt = dist_pool.tile([N, N * R], f32, tag="dist")
                    nflat = dist[nb, :, :, :].rearrange("n j r -> n (j r)")
                    nc.sync.dma_start(out=nt[:, 0:4096], in_=nflat[:, 0:4096])
                    nc.sync.dma_start(out=nt[:, 4096:8192], in_=nflat[:, 4096:8192])
                    dist_tiles[nb] = nt
                g0 = half_fold(dist_t, 0, "a")
                g1 = half_fold(dist_t, 1, "b")
                dsum = dsum_pool.tile([N, R], f32, tag="dsum")
                nc.vector.tensor_add(dsum, g0, g1)
                return dsum

            for p in range(BL // 2):
                b0, b1 = 2 * p, 2 * p + 1
                # d_sum for both batches; transpose to (r, n) and pack pair
                dsT = dsum_pool.tile([R, 2 * N], f32, tag="dsT")
                for k, b in enumerate((b0, b1)):
                    dsum = reduce_j(b)
                    trp = ps1.tile([R, N], f32, tag="tr")
                    nc.tensor.transpose(trp, dsum, ident)
                    nc.scalar.activation(
                        out=dsT[:, k * N : (k + 1) * N], in_=trp, func=AF.Copy
                    )

                xc = wp[:, XOFF + b0 * N : XOFF + (b1 + 1) * N]  # (F, 2N)
                for l in range(L):
                    ms = []
                    for c in range(2):
                        cfp = ps1.tile([128, 2 * N], f32, tag=f"cf{c}")
                        nc.tensor.matmul(cfp, wcf_l(l, c), xc, start=True, stop=True)
                        dfp = ps1.tile([128, 2 * N], f32, tag=f"df{c}")
                        nc.tensor.matmul(dfp, wdf_l(l, c), dsT, start=True, stop=True)
                        cfs = work.tile([128, 2 * N], f32, tag=f"cfs{c}")
                        nc.scalar.activation(out=cfs, in_=cfp, func=AF.Identity, bias=bcf_l(l, c))
                        dfs = work.tile([128, 2 * N], f32, tag=f"dfs{c}")
                        nc.scalar.activation(out=dfs, in_=dfp, func=AF.Identity, bias=bdf_l(l, c))
                        m = work.tile([128, 2 * N], f32, tag=f"m{c}")
                        nc.vector.tensor_mul(m, cfs, dfs)
                        ms.append(m)
                    hp = ps2.tile([F, 2 * N], f32, tag="h")
                    nc.tensor.matmul(hp, wfc_l(l, 0), ms[0], start=True, stop=False)
                    nc.tensor.matmul(hp, wfc_l(l, 1), ms[1], start=False, stop=True)
                    th = work.tile([F, 2 * N], f32, tag="t")
                    nc.scalar.activation(out=th, in_=hp, func=AF.Tanh)
                    xn = work.tile([F, 2 * N], f32, tag="x")
                    nc.vector.tensor_add(xn, hp, th)
                    xc = xn

                # head: out[b] = sum_n sum_f x[f, n] * w_head[f]
                hd = ps1.tile([1, 2 * N], f32, tag="hd")
                nc.tensor.matmul(hd, wp[:, 1548:1549], xc, start=True, stop=True)
                nc.vector.tensor_reduce(
                    out=out_acc[0:1, b0 : b0 + 2],
                    in_=hd.rearrange("o (b n) -> o b n", b=2),
                    axis=AX.X,
                    op=mybir.AluOpType.add,
                )

            nc.sync.dma_start(out=out_ext.rearrange("b o -> o b"), in_=out_acc)

    return nc


def _host_pack(x, Wcf_w, Wcf_b, Wdf_w, Wdf_b, Wfc_w, fc0_w, fc0_b, out_w, out_b):
    f = np.float32
    base = np.zeros((128, WCOLS), f)
    base[:, 0:768] = np.asarray(Wcf_w, f).transpose(2, 0, 1).reshape(128, L * H)
    base[:, 768:1536] = (
        np.asarray(Wfc_w, f).reshape(L, F, 2, 128).transpose(3, 0, 2, 1).reshape(128, L * 2 * F)
    )
    base[:, 1536:1542] = np.asarray(Wcf_b, f).reshape(L, 2, 128).transpose(2, 0, 1).reshape(128, 6)
    base[:, 1542:1548] = (
        (N * np.asarray(Wdf_b, f)).reshape(L, 2, 128).transpose(2, 0, 1).reshape(128, 6)
    )
    w_head = (np.asarray(out_w, np.float64) @ np.asarray(fc0_w, np.float64))[0]  # (F,)
    base[:, 1548] = w_head.astype(f)
    base[0:R, 1552:2320] = np.asarray(Wdf_w, f).transpose(2, 0, 1).reshape(R, L * H)
    base[:, IDOFF : IDOFF + 128] = np.eye(128, dtype=f)

    b_head = float((np.asarray(out_w, np.float64) @ np.asarray(fc0_b, np.float64)
                    + np.asarray(out_b, np.float64)).reshape(()))

    x_t = np.asarray(x, f).transpose(0, 2, 1)  # (B, F, N)
    wpacks = []
    for i in range(NCORES):
        wp = base.copy()
        # wp[f, XOFF + b*N + n] = x_t[global_b, f, n]
        wp[:, XOFF : XOFF + BL * N] = (
            x_t[i * BL : (i + 1) * BL].transpose(1, 0, 2).reshape(128, BL * N)
        )
        wpacks.append(wp)
    return wpacks, b_head


def run(trace=False, **inputs):
    from concourse.bass_utils import run_bass_kernel_spmd

    distance = np.ascontiguousarray(np.asarray(inputs["distance"], np.float32))
    wpacks, b_head = _host_pack(
        inputs["x"], inputs["Wcf_w"], inputs["Wcf_b"], inputs["Wdf_w"], inputs["Wdf_b"],
        inputs["Wfc_w"], inputs["fc0_w"], inputs["fc0_b"], inputs["out_w"], inputs["out_b"],
    )

    if "nc" not in _CACHE:
        nc = _build_program()
        nc.finalize()
        _CACHE["nc"] = nc
    nc = _CACHE["nc"]

    in_maps = []
    for i in range(NCORES):
        in_maps.append({
            "dist": np.ascontiguousarray(distance[i * BL : (i + 1) * BL]),
            "wpack": wpacks[i],
        })
    res = run_bass_kernel_spmd(nc, in_maps, list(range(NCORES)), trace=trace)
    out = np.concatenate([res.results[i]["out"] for i in range(NCORES)], axis=0)
    out = (out.astype(np.float64) + b_head).astype(np.float32)
    return out, res


def kernel(**inputs):
    out, _ = run(trace=False, **inputs)
    return out


# revision 10
# speedup vs baseline: 1.2880x; 1.0043x over previous
"""DeepTensorNeuralNetwork (DTNN / gnn_message_passing) Trainium2 kernel.

Math (per reference):
    d_sum = distance.sum(axis=2)                                  # (B,N,R)
    for l in 0..2:
        cf = x @ Wcf[l].T + bcf[l]                                # (B,N,H)
        df = d_sum @ Wdf[l].T + N*bdf[l]                          # (B,N,H)
        h  = (cf*df) @ Wfc[l].T                                   # (B,N,F)
        x  = h + tanh(h)
    g = x.sum(axis=1); out = (g @ fc0.T + b0) @ ow.T + ob         # (B,1)

Strategy: data-parallel over batch across 8 NeuronCores (8 batches each).
The dominant cost is streaming `distance` (33.5 MB/core) from HBM at
~358 GB/s (~94us floor). The j-reduction is a DVE binary fold tree done
in-place (measured: tensor_reduce is 1.64 cyc/elem, folds are 1
cyc/output; GpSimd "help" serializes against DVE on the shared SBUF
port, so DVE does everything). Each batch's DMA is split into j-halves
so folding starts after the first half lands. Layer matmuls process
FOUR batches at once (moving dim N=512 = fp32 max) on the PE in fp32;
ACT does PSUM->SBUF bias copies and tanh; DVE does cf*df and the
residual add. The affine head (fc0 + out) is folded on the host into a
single length-F vector + scalar bias (mathematically identical). All
constants ship in ONE packed array -> one DMA -> one wait sem.
"""

import numpy as np

B, N, F, R, H = 64, 128, 128, 64, 256
L = 3
NCORES = 8
BL = B // NCORES  # batches per core
G = 4             # batches per layer-compute group (moving dim G*N = 512)

# wpack column layout (128 partitions):
#   [0, 768)       wcf lhsT   : wpack[f, l*H + h]          = Wcf_w[l, h, f]
#   [768, 1536)    wfc lhsT   : wpack[hc, 768+(l*2+c)*F+f] = Wfc_w[l, f, c*128+hc]
#   [1536, 1542)   cf bias    : wpack[h, 1536 + l*2+c]     = Wcf_b[l, c*128+h]
#   [1542, 1548)   df bias    : wpack[h, 1542 + l*2+c]     = N * Wdf_b[l, c*128+h]
#   [1548]         head lhsT  : wpack[f, 1548]             = (out_w @ fc0_w)[0, f]
#   [1552, 2320)   wdf lhsT   : wpack[r, 1552 + l*H + h]   = Wdf_w[l, h, r]   (rows 0:64)
#   [2320, 3344)   x          : wpack[f, 2320 + b*N + n]   = x[b_local, n, f]
#   [3344, 3472)   identity 128x128
XOFF = 2320
IDOFF = 3344
WCOLS = 3472

_CACHE = {}


def _build_program():
    import concourse.bass as bass
    from concourse import bacc
    import concourse.tile as tile
    from concourse import mybir

    f32 = mybir.dt.float32
    AX = mybir.AxisListType
    AF = mybir.ActivationFunctionType

    nc = bacc.Bacc("TRN2")
    dist = nc.declare_dram_parameter("dist", [BL, N, N, R], f32, isOutput=False)
    wpack = nc.declare_dram_parameter("wpack", [128, WCOLS], f32, isOutput=False)
    out_ext = nc.declare_dram_parameter("out", [BL, 1], f32, isOutput=True)

    with tile.TileContext(nc) as tc:
        with (
            tc.tile_pool(name="consts", bufs=1) as consts,
            tc.tile_pool(name="dist", bufs=3) as dist_pool,
            tc.tile_pool(name="fold", bufs=2) as fold_pool,
            tc.tile_pool(name="dsum", bufs=2) as dsum_pool,
            tc.tile_pool(name="work", bufs=2) as work,
            tc.tile_pool(name="ps1", bufs=1, space="PSUM") as ps1,
            tc.tile_pool(name="ps2", bufs=2, space="PSUM") as ps2,
        ):
            # issue the first distance loads BEFORE the weight pack so the
            # fold pipeline starts as early as possible (Sync queue is FIFO)
            dist_tiles = {}

            def start_dist_dma(b):
                t = dist_pool.tile([N, N * R], f32, tag="dist")
                dflat = dist[b, :, :, :].rearrange("n j r -> n (j r)")
                nc.sync.dma_start(out=t[:, 0:4096], in_=dflat[:, 0:4096])
                nc.sync.dma_start(out=t[:, 4096:8192], in_=dflat[:, 4096:8192])
                dist_tiles[b] = t

            for b in range(2):
                start_dist_dma(b)

            wp = consts.tile([128, WCOLS], f32)
            nc.sync.dma_start(out=wp, in_=wpack[:, :])
            ident = wp[:, IDOFF : IDOFF + 128]
            out_acc = consts.tile([1, BL], f32)

            def wcf_l(l, c):
                return wp[:, l * H + c * 128 : l * H + (c + 1) * 128]

            def wdf_l(l, c):
                o = 1552 + l * H + c * 128
                return wp[0:R, o : o + 128]

            def wfc_l(l, c):
                o = 768 + (l * 2 + c) * F
                return wp[:, o : o + F]

            def bcf_l(l, c):
                o = 1536 + l * 2 + c
                return wp[:, o : o + 1]

            def bdf_l(l, c):
                o = 1542 + l * 2 + c
                return wp[:, o : o + 1]

            def half_fold(src, half, tag_suffix):
                """DVE fold tree over one j-half: 4096 elems/lane -> 64.
                Ping-pong between two scratch tiles."""
                o = half * 4096
                s = fold_pool.tile([N, 2048], f32, tag=f"s{tag_suffix}")
                nc.vector.tensor_add(s, src[:, o : o + 2048], src[:, o + 2048 : o + 4096])
                t = fold_pool.tile([N, 1024], f32, tag=f"t{tag_suffix}")
                cur, other, w = s, t, 1024
                while w >= 64:
                    nc.vector.tensor_add(other[:, 0:w], cur[:, 0:w], cur[:, w : 2 * w])
                    cur, other = other, cur
                    w //= 2
                return cur

            def reduce_j(b):
                dist_t = dist_tiles.pop(b)
                if b + 2 < BL:  # keep two batches in flight
                    start_dist_dma(b + 2)
                g0 = half_fold(dist_t, 0, "a")
                g1 = half_fold(dist_t, 1, "b")
                dsum = dsum_pool.tile([N, R], f32, tag="dsum")
                nc.vector.tensor_add(dsum, g0[:, 0:64], g1[:, 0:64])
                return dsum

            NG = G * N  # moving dim of layer matmuls
            for q in range(BL // G):
                bs = list(range(q * G, (q + 1) * G))
                # d_sum for the group's batches; transpose to (r, n); pack
                dsT = dsum_pool.tile([R, NG], f32, tag="dsT")
                for k, b in enumerate(bs):
                    dsum = reduce_j(b)
                    trp = ps1.tile([R, N], f32, tag="tr")
                    nc.tensor.transpose(trp, dsum, ident)
                    nc.scalar.activation(
                        out=dsT[:, k * N : (k + 1) * N], in_=trp, func=AF.Copy
                    )

                xc = wp[:, XOFF + bs[0] * N : XOFF + (bs[-1] + 1) * N]  # (F, NG)
                for l in range(L):
                    ms = []
                    for c in range(2):
                        cfp = ps1.tile([128, NG], f32, tag=f"cf{c}")
                        nc.tensor.matmul(cfp, wcf_l(l, c), xc, start=True, stop=True)
                        dfp = ps1.tile([128, NG], f32, tag=f"df{c}")
                        nc.tensor.matmul(dfp, wdf_l(l, c), dsT, start=True, stop=True)
                        cfs = work.tile([128, NG], f32, tag=f"cfs{c}")
                        nc.scalar.activation(out=cfs, in_=cfp, func=AF.Identity, bias=bcf_l(l, c))
                        dfs = work.tile([128, NG], f32, tag=f"dfs{c}")
                        nc.scalar.activation(out=dfs, in_=dfp, func=AF.Identity, bias=bdf_l(l, c))
                        m = work.tile([128, NG], f32, tag=f"m{c}")
                        nc.vector.tensor_mul(m, cfs, dfs)
                        ms.append(m)
                    hp = ps2.tile([F, NG], f32, tag="h")
                    nc.tensor.matmul(hp, wfc_l(l, 0), ms[0], start=True, stop=False)
                    nc.tensor.matmul(hp, wfc_l(l, 1), ms[1], start=False, stop=True)
                    th = work.tile([F, NG], f32, tag="t")
                    nc.scalar.activation(out=th, in_=hp, func=AF.Tanh)
                    xn = work.tile([F, NG], f32, tag="x")
                    nc.vector.tensor_add(xn, hp, th)
                    xc = xn

                # head: out[b] = sum_n sum_f x[f, n] * w_head[f]
                hd = ps1.tile([1, NG], f32, tag="hd")
                nc.tensor.matmul(hd, wp[:, 1548:1549], xc, start=True, stop=True)
                nc.vector.tensor_reduce(
                    out=out_acc[0:1, bs[0] : bs[0] + G],
                    in_=hd.rearrange("o (b n) -> o b n", b=G),
                    axis=AX.X,
                    op=mybir.AluOpType.add,
                )

            nc.sync.dma_start(out=out_ext.rearrange("b o -> o b"), in_=out_acc)

    return nc


def _host_pack(x, Wcf_w, Wcf_b, Wdf_w, Wdf_b, Wfc_w, fc0_w, fc0_b, out_w, out_b):
    f = np.float32
    base = np.zeros((128, WCOLS), f)
    base[:, 0:768] = np.asarray(Wcf_w, f).transpose(2, 0, 1).reshape(128, L * H)
    base[:, 768:1536] = (
        np.asarray(Wfc_w, f).reshape(L, F, 2, 128).transpose(3, 0, 2, 1).reshape(128, L * 2 * F)
    )
    base[:, 1536:1542] = np.asarray(Wcf_b, f).reshape(L, 2, 128).transpose(2, 0, 1).reshape(128, 6)
    base[:, 1542:1548] = (
        (N * np.asarray(Wdf_b, f)).reshape(L, 2, 128).transpose(2, 0, 1).reshape(128, 6)
    )
    w_head = (np.asarray(out_w, np.float64) @ np.asarray(fc0_w, np.float64))[0]  # (F,)
    base[:, 1548] = w_head.astype(f)
    base[0:R, 1552:2320] = np.asarray(Wdf_w, f).transpose(2, 0, 1).reshape(R, L * H)
    base[:, IDOFF : IDOFF + 128] = np.eye(128, dtype=f)

    b_head = float((np.asarray(out_w, np.float64) @ np.asarray(fc0_b, np.float64)
                    + np.asarray(out_b, np.float64)).reshape(()))

    x_t = np.asarray(x, f).transpose(0, 2, 1)  # (B, F, N)
    wpacks = []
    for i in range(NCORES):
        wp = base.copy()
        # wp[f, XOFF + b*N + n] = x_t[global_b, f, n]
        wp[:, XOFF : XOFF + BL * N] = (
            x_t[i * BL : (i + 1) * BL].transpose(1, 0, 2).reshape(128, BL * N)
        )
        wpacks.append(wp)
    return wpacks, b_head


def run(trace=False, **inputs):
    from concourse.bass_utils import run_bass_kernel_spmd

    distance = np.ascontiguousarray(np.asarray(inputs["distance"], np.float32))
    wpacks, b_head = _host_pack(
        inputs["x"], inputs["Wcf_w"], inputs["Wcf_b"], inputs["Wdf_w"], inputs["Wdf_b"],
        inputs["Wfc_w"], inputs["fc0_w"], inputs["fc0_b"], inputs["out_w"], inputs["out_b"],
    )

    if "nc" not in _CACHE:
        nc = _build_program()
        nc.finalize()
        _CACHE["nc"] = nc
    nc = _CACHE["nc"]

    in_maps = []
    for i in range(NCORES):
        in_maps.append({
            "dist": np.ascontiguousarray(distance[i * BL : (i + 1) * BL]),
            "wpack": wpacks[i],
        })
    res = run_bass_kernel_spmd(nc, in_maps, list(range(NCORES)), trace=trace)
    out = np.concatenate([res.results[i]["out"] for i in range(NCORES)], axis=0)
    out = (out.astype(np.float64) + b_head).astype(np.float32)
    return out, res


def kernel(**inputs):
    out, _ = run(trace=False, **inputs)
    return out


# revision 12
# speedup vs baseline: 1.5452x; 1.1996x over previous
"""DeepTensorNeuralNetwork (DTNN / gnn_message_passing) Trainium2 kernel.

Math (per reference):
    d_sum = distance.sum(axis=2)                                  # (B,N,R)
    for l in 0..2:
        cf = x @ Wcf[l].T + bcf[l]                                # (B,N,H)
        df = d_sum @ Wdf[l].T + N*bdf[l]                          # (B,N,H)
        h  = (cf*df) @ Wfc[l].T                                   # (B,N,F)
        x  = h + tanh(h)
    g = x.sum(axis=1); out = (g @ fc0.T + b0) @ ow.T + ob         # (B,1)

Strategy: data-parallel over batch across 8 NeuronCores (8 batches each).
The dominant cost is streaming `distance` (33.5 MB/core) from HBM at
~358 GB/s (~94us floor). The j-reduction is an fp32 DVE binary fold
tree (measured: tensor_reduce is 1.64 cyc/elem, folds are 1 cyc/output;
GpSimd "help" serializes against DVE on the shared SBUF port, so DVE
does everything). Each batch's DMA is split into j-halves so folding
starts after the first half lands. The layer pipeline runs in bf16
(inputs rounded to bf16; PSUM accumulation stays fp32): matmuls process
batch GROUPS (4,2,2 - the last groups are small to shorten the tail
chain) on the PE; ACT does PSUM->SBUF bias copies and tanh; DVE does
cf*df and the residual add. The affine head (fc0 + out) is folded on
the host into a single length-F vector + scalar bias. All constants
ship in ONE packed array -> one DMA -> one wait sem.
"""

import numpy as np

B, N, F, R, H = 64, 128, 128, 64, 256
L = 3
NCORES = 8
BL = B // NCORES   # batches per core
GROUPS = (4, 2, 2)  # batches per layer-compute group

# wpack layout, fp32 columns (bf16 sections hold 2 bf16 per column):
#   [0, 384)      wcf lhsT bf16 : bf-col l*H+h           = Wcf_w[l, h, f]
#   [384, 768)    wfc lhsT bf16 : bf-col (l*2+c)*F+f     = Wfc_w[l, f, c*128+hc]
#   [768, 774)    cf bias fp32  : col l*2+c              = Wcf_b[l, c*128+h]
#   [774, 780)    df bias fp32  : col l*2+c              = N * Wdf_b[l, c*128+h]
#   [780, 781)    head lhsT bf16: bf-col 0               = (out_w @ fc0_w)[0, f]
#   [784, 1168)   wdf lhsT bf16 : bf-col l*H+h (rows<64) = Wdf_w[l, h, r]
#   [1168, 1680)  x bf16        : bf-col b*N+n           = x[b_local, n, f]
#   [1680, 1808)  identity fp32
BCF_OFF = 768
BDF_OFF = 774
HEAD_OFF = 780
WDF_OFF = 784
XOFF = 1168
IDOFF = 1680
WCOLS = 1808

_CACHE = {}


def _build_program():
    import concourse.bass as bass
    from concourse import bacc
    import concourse.tile as tile
    from concourse import mybir

    f32 = mybir.dt.float32
    bf16 = mybir.dt.bfloat16
    AX = mybir.AxisListType
    AF = mybir.ActivationFunctionType

    nc = bacc.Bacc("TRN2")
    dist = nc.declare_dram_parameter("dist", [BL, N, N, R], f32, isOutput=False)
    wpack = nc.declare_dram_parameter("wpack", [128, WCOLS], f32, isOutput=False)
    out_ext = nc.declare_dram_parameter("out", [BL, 1], f32, isOutput=True)

    with tile.TileContext(nc) as tc:
        with (
            tc.tile_pool(name="consts", bufs=1) as consts,
            tc.tile_pool(name="dist", bufs=3) as dist_pool,
            tc.tile_pool(name="fold", bufs=2) as fold_pool,
            tc.tile_pool(name="dsum", bufs=2) as dsum_pool,
            tc.tile_pool(name="work", bufs=2) as work,
            tc.tile_pool(name="ps1", bufs=1, space="PSUM") as ps1,
            tc.tile_pool(name="ps2", bufs=2, space="PSUM") as ps2,
        ):
            # issue the first distance loads BEFORE the weight pack so the
            # fold pipeline starts as early as possible (Sync queue is FIFO)
            dist_tiles = {}

            def start_dist_dma(b):
                t = dist_pool.tile([N, N * R], f32, tag="dist")
                dflat = dist[b, :, :, :].rearrange("n j r -> n (j r)")
                nc.sync.dma_start(out=t[:, 0:4096], in_=dflat[:, 0:4096])
                nc.sync.dma_start(out=t[:, 4096:8192], in_=dflat[:, 4096:8192])
                dist_tiles[b] = t

            for b in range(2):
                start_dist_dma(b)

            wp = consts.tile([128, WCOLS], f32)
            nc.sync.dma_start(out=wp, in_=wpack[:, :])
            wb = wp.bitcast(bf16)  # (128, 2*WCOLS) bf16 view
            ident = wp[:, IDOFF : IDOFF + 128]
            out_acc = consts.tile([1, BL], f32)

            def wcf_l(l, c):
                o = l * H + c * 128
                return wb[:, o : o + 128]

            def wdf_l(l, c):
                o = 2 * WDF_OFF + l * H + c * 128
                return wb[0:R, o : o + 128]

            def wfc_l(l, c):
                o = 2 * 384 + (l * 2 + c) * F
                return wb[:, o : o + F]

            def bcf_l(l, c):
                o = BCF_OFF + l * 2 + c
                return wp[:, o : o + 1]

            def bdf_l(l, c):
                o = BDF_OFF + l * 2 + c
                return wp[:, o : o + 1]

            def half_fold(src, half, tag_suffix):
                """DVE fold tree over one j-half: 4096 elems/lane -> 64.
                Ping-pong between two scratch tiles."""
                o = half * 4096
                s = fold_pool.tile([N, 2048], f32, tag=f"s{tag_suffix}")
                nc.vector.tensor_add(s, src[:, o : o + 2048], src[:, o + 2048 : o + 4096])
                t = fold_pool.tile([N, 1024], f32, tag=f"t{tag_suffix}")
                cur, other, w = s, t, 1024
                while w >= 64:
                    nc.vector.tensor_add(other[:, 0:w], cur[:, 0:w], cur[:, w : 2 * w])
                    cur, other = other, cur
                    w //= 2
                return cur

            def reduce_j(b):
                dist_t = dist_tiles.pop(b)
                if b + 2 < BL:  # keep two batches in flight
                    start_dist_dma(b + 2)
                g0 = half_fold(dist_t, 0, "a")
                g1 = half_fold(dist_t, 1, "b")
                dsum = dsum_pool.tile([N, R], f32, tag="dsum")
                nc.vector.tensor_add(dsum, g0[:, 0:64], g1[:, 0:64])
                return dsum

            b0 = 0
            for G in GROUPS:
                bs = list(range(b0, b0 + G))
                b0 += G
                NG = G * N
                # d_sum for the group's batches; transpose to (r, n); pack bf16
                dsT = dsum_pool.tile([R, 4 * N], bf16, tag="dsT")
                for k, b in enumerate(bs):
                    dsum = reduce_j(b)
                    trp = ps1.tile([R, N], f32, tag="tr")
                    nc.tensor.transpose(trp, dsum, ident)
                    nc.scalar.activation(
                        out=dsT[:, k * N : (k + 1) * N], in_=trp, func=AF.Copy
                    )

                xc = wb[:, 2 * XOFF + bs[0] * N : 2 * XOFF + (bs[-1] + 1) * N]  # (F, NG) bf16
                for l in range(L):
                    ms = []
                    for c in range(2):
                        cfp = ps1.tile([128, 4 * N], f32, tag=f"cf{c}", name=f"cfp{c}")[:, 0:NG]
                        nc.tensor.matmul(cfp, wcf_l(l, c), xc, start=True, stop=True)
                        dfp = ps1.tile([128, 4 * N], f32, tag=f"df{c}", name=f"dfp{c}")[:, 0:NG]
                        nc.tensor.matmul(dfp, wdf_l(l, c), dsT[:, 0:NG], start=True, stop=True)
                        cfs = work.tile([128, 4 * N], bf16, tag=f"cfs{c}", name=f"cfs{c}")[:, 0:NG]
                        nc.scalar.activation(out=cfs, in_=cfp, func=AF.Identity, bias=bcf_l(l, c))
                        dfs = work.tile([128, 4 * N], bf16, tag=f"dfs{c}", name=f"dfs{c}")[:, 0:NG]
                        nc.scalar.activation(out=dfs, in_=dfp, func=AF.Identity, bias=bdf_l(l, c))
                        m = work.tile([128, 4 * N], bf16, tag=f"m{c}", name=f"m{c}")[:, 0:NG]
                        nc.vector.tensor_mul(m, cfs, dfs)
                        ms.append(m)
                    hp = ps2.tile([F, 4 * N], f32, tag="h", name="hp")[:, 0:NG]
                    nc.tensor.matmul(hp, wfc_l(l, 0), ms[0], start=True, stop=False)
                    nc.tensor.matmul(hp, wfc_l(l, 1), ms[1], start=False, stop=True)
                    th = work.tile([F, 4 * N], f32, tag="t", name="th")[:, 0:NG]
                    nc.scalar.activation(out=th, in_=hp, func=AF.Tanh)
                    xn = work.tile([F, 4 * N], bf16, tag="x", name="xn")[:, 0:NG]
                    nc.vector.tensor_add(xn, hp, th)
                    xc = xn

                # head: out[b] = sum_n sum_f x[f, n] * w_head[f]
                hd = ps1.tile([1, 4 * N], f32, tag="hd", name="hd")[:, 0:NG]
                nc.tensor.matmul(hd, wb[:, 2 * HEAD_OFF : 2 * HEAD_OFF + 1], xc,
                                 start=True, stop=True)
                nc.vector.tensor_reduce(
                    out=out_acc[0:1, bs[0] : bs[0] + G],
                    in_=hd.rearrange("o (b n) -> o b n", b=G),
                    axis=AX.X,
                    op=mybir.AluOpType.add,
                )

            nc.sync.dma_start(out=out_ext.rearrange("b o -> o b"), in_=out_acc)

    return nc


def _host_pack(x, Wcf_w, Wcf_b, Wdf_w, Wdf_b, Wfc_w, fc0_w, fc0_b, out_w, out_b):
    import ml_dtypes

    f = np.float32
    bf = ml_dtypes.bfloat16

    def pack_bf(a):  # (128, 2K) bf16 -> (128, K) fp32 bit-packed
        return np.ascontiguousarray(a.astype(bf)).view(f)

    base = np.zeros((128, WCOLS), f)
    base[:, 0:384] = pack_bf(np.asarray(Wcf_w, f).transpose(2, 0, 1).reshape(128, L * H))
    base[:, 384:768] = pack_bf(
        np.asarray(Wfc_w, f).reshape(L, F, 2, 128).transpose(3, 0, 2, 1).reshape(128, L * 2 * F)
    )
    base[:, BCF_OFF : BCF_OFF + 6] = (
        np.asarray(Wcf_b, f).reshape(L, 2, 128).transpose(2, 0, 1).reshape(128, 6)
    )
    base[:, BDF_OFF : BDF_OFF + 6] = (
        (N * np.asarray(Wdf_b, f)).reshape(L, 2, 128).transpose(2, 0, 1).reshape(128, 6)
    )
    w_head = (np.asarray(out_w, np.float64) @ np.asarray(fc0_w, np.float64))[0]  # (F,)
    head_pair = np.zeros((128, 2), f)
    head_pair[:, 0] = w_head.astype(f)
    base[:, HEAD_OFF : HEAD_OFF + 1] = pack_bf(head_pair)
    base[0:R, WDF_OFF : WDF_OFF + 384] = pack_bf(
        np.asarray(Wdf_w, f).transpose(2, 0, 1).reshape(R, L * H)
    )
    base[:, IDOFF : IDOFF + 128] = np.eye(128, dtype=f)

    b_head = float((np.asarray(out_w, np.float64) @ np.asarray(fc0_b, np.float64)
                    + np.asarray(out_b, np.float64)).reshape(()))

    x_t = np.asarray(x, f).transpose(0, 2, 1)  # (B, F, N)
    wpacks = []
    for i in range(NCORES):
        wp = base.copy()
        wp[:, XOFF : XOFF + BL * N // 2] = pack_bf(
            x_t[i * BL : (i + 1) * BL].transpose(1, 0, 2).reshape(128, BL * N)
        )
        wpacks.append(wp)
    return wpacks, b_head


def run(trace=False, **inputs):
    from concourse.bass_utils import run_bass_kernel_spmd

    distance = np.ascontiguousarray(np.asarray(inputs["distance"], np.float32))
    wpacks, b_head = _host_pack(
        inputs["x"], inputs["Wcf_w"], inputs["Wcf_b"], inputs["Wdf_w"], inputs["Wdf_b"],
        inputs["Wfc_w"], inputs["fc0_w"], inputs["fc0_b"], inputs["out_w"], inputs["out_b"],
    )

    if "nc" not in _CACHE:
        nc = _build_program()
        nc.finalize()
        _CACHE["nc"] = nc
    nc = _CACHE["nc"]

    in_maps = []
    for i in range(NCORES):
        in_maps.append({
            "dist": np.ascontiguousarray(distance[i * BL : (i + 1) * BL]),
            "wpack": wpacks[i],
        })
    res = run_bass_kernel_spmd(nc, in_maps, list(range(NCORES)), trace=trace)
    out = np.concatenate([res.results[i]["out"] for i in range(NCORES)], axis=0)
    out = (out.astype(np.float64) + b_head).astype(np.float32)
    return out, res


def kernel(**inputs):
    out, _ = run(trace=False, **inputs)
    return out


# revision 13
# speedup vs baseline: 1.6559x; 1.0717x over previous
"""DeepTensorNeuralNetwork (DTNN / gnn_message_passing) Trainium2 kernel.

Math (per reference):
    d_sum = distance.sum(axis=2)                                  # (B,N,R)
    for l in 0..2:
        cf = x @ Wcf[l].T + bcf[l]                                # (B,N,H)
        df = d_sum @ Wdf[l].T + N*bdf[l]                          # (B,N,H)
        h  = (cf*df) @ Wfc[l].T                                   # (B,N,F)
        x  = h + tanh(h)
    g = x.sum(axis=1); out = (g @ fc0.T + b0) @ ow.T + ob         # (B,1)

Strategy: data-parallel over batch across 8 NeuronCores (8 batches each).
The dominant cost is streaming `distance` (33.5 MB/core) from HBM at
~358 GB/s (~94us floor). The j-reduction is an fp32 DVE binary fold
tree (measured: tensor_reduce is 1.64 cyc/elem, folds are 1 cyc/output;
GpSimd "help" serializes against DVE on the shared SBUF port, so DVE
does everything). Each batch's DMA is split into j-halves so folding
starts after the first half lands. The layer pipeline runs in bf16
(inputs rounded to bf16; PSUM accumulation stays fp32): matmuls process
batch GROUPS (4,2,2 - the last groups are small to shorten the tail
chain) on the PE; ACT does PSUM->SBUF bias copies and tanh; DVE does
cf*df and the residual add. The affine head (fc0 + out) is folded on
the host into a single length-F vector + scalar bias. All constants
ship in ONE packed array -> one DMA -> one wait sem.
"""

import numpy as np

B, N, F, R, H = 64, 128, 128, 64, 256
L = 3
NCORES = 8
BL = B // NCORES   # batches per core
GROUPS = (4, 2, 2)  # batches per layer-compute group

# wpack layout, fp32 columns (fp16 sections hold 2 fp16 per column):
#   [0, 384)      wcf lhsT bf16 : bf-col l*H+h           = Wcf_w[l, h, f]
#   [384, 768)    wfc lhsT bf16 : bf-col (l*2+c)*F+f     = Wfc_w[l, f, c*128+hc]
#   [768, 774)    cf bias fp32  : col l*2+c              = Wcf_b[l, c*128+h]
#   [774, 780)    df bias fp32  : col l*2+c              = N * Wdf_b[l, c*128+h]
#   [780, 781)    head lhsT bf16: bf-col 0               = (out_w @ fc0_w)[0, f]
#   [784, 1168)   wdf lhsT bf16 : bf-col l*H+h (rows<64) = Wdf_w[l, h, r]
#   [1168, 1680)  x bf16        : bf-col b*N+n           = x[b_local, n, f]
#   [1680, 1808)  identity fp32
BCF_OFF = 768
BDF_OFF = 774
HEAD_OFF = 780
WDF_OFF = 784
XOFF = 1168
IDOFF = 1680
WCOLS = 1808

_CACHE = {}


def _build_program():
    import concourse.bass as bass
    from concourse import bacc
    import concourse.tile as tile
    from concourse import mybir

    f32 = mybir.dt.float32
    bf16 = mybir.dt.float16
    AX = mybir.AxisListType
    AF = mybir.ActivationFunctionType

    nc = bacc.Bacc("TRN2")
    dist = nc.declare_dram_parameter("dist", [BL, N, N, R], f32, isOutput=False)
    wpack = nc.declare_dram_parameter("wpack", [128, WCOLS], f32, isOutput=False)
    out_ext = nc.declare_dram_parameter("out", [BL, 1], f32, isOutput=True)

    with tile.TileContext(nc) as tc:
        with (
            tc.tile_pool(name="consts", bufs=1) as consts,
            tc.tile_pool(name="dist", bufs=3) as dist_pool,
            tc.tile_pool(name="fold", bufs=2) as fold_pool,
            tc.tile_pool(name="dsum", bufs=2) as dsum_pool,
            tc.tile_pool(name="work", bufs=2) as work,
            tc.tile_pool(name="ps1", bufs=1, space="PSUM") as ps1,
            tc.tile_pool(name="ps2", bufs=2, space="PSUM") as ps2,
        ):
            # issue the first distance loads BEFORE the weight pack so the
            # fold pipeline starts as early as possible (Sync queue is FIFO)
            dist_tiles = {}

            def start_dist_dma(b):
                t = dist_pool.tile([N, N * R], f32, tag="dist")
                dflat = dist[b, :, :, :].rearrange("n j r -> n (j r)")
                nc.sync.dma_start(out=t[:, 0:4096], in_=dflat[:, 0:4096])
                nc.sync.dma_start(out=t[:, 4096:8192], in_=dflat[:, 4096:8192])
                dist_tiles[b] = t

            for b in range(2):
                start_dist_dma(b)

            wp = consts.tile([128, WCOLS], f32)
            nc.sync.dma_start(out=wp, in_=wpack[:, :])
            wb = wp.bitcast(bf16)  # (128, 2*WCOLS) bf16 view
            ident = wp[:, IDOFF : IDOFF + 128]
            out_acc = consts.tile([1, BL], f32)

            def wcf_l(l, c):
                o = l * H + c * 128
                return wb[:, o : o + 128]

            def wdf_l(l, c):
                o = 2 * WDF_OFF + l * H + c * 128
                return wb[0:R, o : o + 128]

            def wfc_l(l, c):
                o = 2 * 384 + (l * 2 + c) * F
                return wb[:, o : o + F]

            def bcf_l(l, c):
                o = BCF_OFF + l * 2 + c
                return wp[:, o : o + 1]

            def bdf_l(l, c):
                o = BDF_OFF + l * 2 + c
                return wp[:, o : o + 1]

            def half_fold(src, half, tag_suffix):
                """DVE fold tree over one j-half: 4096 elems/lane -> 64.
                Ping-pong between two scratch tiles."""
                o = half * 4096
                s = fold_pool.tile([N, 2048], f32, tag=f"s{tag_suffix}")
                nc.vector.tensor_add(s, src[:, o : o + 2048], src[:, o + 2048 : o + 4096])
                t = fold_pool.tile([N, 1024], f32, tag=f"t{tag_suffix}")
                cur, other, w = s, t, 1024
                while w >= 64:
                    nc.vector.tensor_add(other[:, 0:w], cur[:, 0:w], cur[:, w : 2 * w])
                    cur, other = other, cur
                    w //= 2
                return cur

            def reduce_j(b):
                dist_t = dist_tiles.pop(b)
                if b + 2 < BL:  # keep two batches in flight
                    start_dist_dma(b + 2)
                g0 = half_fold(dist_t, 0, "a")
                g1 = half_fold(dist_t, 1, "b")
                dsum = dsum_pool.tile([N, R], f32, tag="dsum")
                nc.vector.tensor_add(dsum, g0[:, 0:64], g1[:, 0:64])
                return dsum

            b0 = 0
            for G in GROUPS:
                bs = list(range(b0, b0 + G))
                b0 += G
                NG = G * N
                # d_sum for the group's batches; transpose to (r, n); pack bf16
                dsT = dsum_pool.tile([R, 4 * N], bf16, tag="dsT")
                for k, b in enumerate(bs):
                    dsum = reduce_j(b)
                    trp = ps1.tile([R, N], f32, tag="tr")
                    nc.tensor.transpose(trp, dsum, ident)
                    nc.scalar.activation(
                        out=dsT[:, k * N : (k + 1) * N], in_=trp, func=AF.Copy
                    )

                xc = wb[:, 2 * XOFF + bs[0] * N : 2 * XOFF + (bs[-1] + 1) * N]  # (F, NG) bf16
                for l in range(L):
                    ms = []
                    for c in range(2):
                        cfp = ps1.tile([128, 4 * N], f32, tag=f"cf{c}", name=f"cfp{c}")[:, 0:NG]
                        nc.tensor.matmul(cfp, wcf_l(l, c), xc, start=True, stop=True)
                        dfp = ps1.tile([128, 4 * N], f32, tag=f"df{c}", name=f"dfp{c}")[:, 0:NG]
                        nc.tensor.matmul(dfp, wdf_l(l, c), dsT[:, 0:NG], start=True, stop=True)
                        cfs = work.tile([128, 4 * N], bf16, tag=f"cfs{c}", name=f"cfs{c}")[:, 0:NG]
                        nc.scalar.activation(out=cfs, in_=cfp, func=AF.Identity, bias=bcf_l(l, c))
                        dfs = work.tile([128, 4 * N], bf16, tag=f"dfs{c}", name=f"dfs{c}")[:, 0:NG]
                        nc.scalar.activation(out=dfs, in_=dfp, func=AF.Identity, bias=bdf_l(l, c))
                        m = work.tile([128, 4 * N], bf16, tag=f"m{c}", name=f"m{c}")[:, 0:NG]
                        nc.vector.tensor_mul(m, cfs, dfs)
                        ms.append(m)
                    hp = ps2.tile([F, 4 * N], f32, tag="h", name="hp")[:, 0:NG]
                    nc.tensor.matmul(hp, wfc_l(l, 0), ms[0], start=True, stop=False)
                    nc.tensor.matmul(hp, wfc_l(l, 1), ms[1], start=False, stop=True)
                    th = work.tile([F, 4 * N], f32, tag="t", name="th")[:, 0:NG]
                    nc.scalar.activation(out=th, in_=hp, func=AF.Tanh)
                    xn = work.tile([F, 4 * N], bf16, tag="x", name="xn")[:, 0:NG]
                    nc.vector.tensor_add(xn, hp, th)
                    xc = xn

                # head: out[b] = sum_n sum_f x[f, n] * w_head[f]
                hd = ps1.tile([1, 4 * N], f32, tag="hd", name="hd")[:, 0:NG]
                nc.tensor.matmul(hd, wb[:, 2 * HEAD_OFF : 2 * HEAD_OFF + 1], xc,
                                 start=True, stop=True)
                nc.vector.tensor_reduce(
                    out=out_acc[0:1, bs[0] : bs[0] + G],
                    in_=hd.rearrange("o (b n) -> o b n", b=G),
                    axis=AX.X,
                    op=mybir.AluOpType.add,
                )

            nc.sync.dma_start(out=out_ext.rearrange("b o -> o b"), in_=out_acc)

    return nc


def _host_pack(x, Wcf_w, Wcf_b, Wdf_w, Wdf_b, Wfc_w, fc0_w, fc0_b, out_w, out_b):
    import ml_dtypes

    f = np.float32
    bf = np.float16

    def pack_bf(a):  # (128, 2K) bf16 -> (128, K) fp32 bit-packed
        return np.ascontiguousarray(a.astype(bf)).view(f)

    base = np.zeros((128, WCOLS), f)
    base[:, 0:384] = pack_bf(np.asarray(Wcf_w, f).transpose(2, 0, 1).reshape(128, L * H))
    base[:, 384:768] = pack_bf(
        np.asarray(Wfc_w, f).reshape(L, F, 2, 128).transpose(3, 0, 2, 1).reshape(128, L * 2 * F)
    )
    base[:, BCF_OFF : BCF_OFF + 6] = (
        np.asarray(Wcf_b, f).reshape(L, 2, 128).transpose(2, 0, 1).reshape(128, 6)
    )
    base[:, BDF_OFF : BDF_OFF + 6] = (
        (N * np.asarray(Wdf_b, f)).reshape(L, 2, 128).transpose(2, 0, 1).reshape(128, 6)
    )
    w_head = (np.asarray(out_w, np.float64) @ np.asarray(fc0_w, np.float64))[0]  # (F,)
    head_pair = np.zeros((128, 2), f)
    head_pair[:, 0] = w_head.astype(f)
    base[:, HEAD_OFF : HEAD_OFF + 1] = pack_bf(head_pair)
    base[0:R, WDF_OFF : WDF_OFF + 384] = pack_bf(
        np.asarray(Wdf_w, f).transpose(2, 0, 1).reshape(R, L * H)
    )
    base[:, IDOFF : IDOFF + 128] = np.eye(128, dtype=f)

    b_head = float((np.asarray(out_w, np.float64) @ np.asarray(fc0_b, np.float64)
                    + np.asarray(out_b, np.float64)).reshape(()))

    x_t = np.asarray(x, f).transpose(0, 2, 1)  # (B, F, N)
    wpacks = []
    for i in range(NCORES):
        wp = base.copy()
        wp[:, XOFF : XOFF + BL * N // 2] = pack_bf(
            x_t[i * BL : (i + 1) * BL].transpose(1, 0, 2).reshape(128, BL * N)
        )
        wpacks.append(wp)
    return wpacks, b_head


def run(trace=False, **inputs):
    from concourse.bass_utils import run_bass_kernel_spmd

    distance = np.ascontiguousarray(np.asarray(inputs["distance"], np.float32))
    wpacks, b_head = _host_pack(
        inputs["x"], inputs["Wcf_w"], inputs["Wcf_b"], inputs["Wdf_w"], inputs["Wdf_b"],
        inputs["Wfc_w"], inputs["fc0_w"], inputs["fc0_b"], inputs["out_w"], inputs["out_b"],
    )

    if "nc" not in _CACHE:
        nc = _build_program()
        nc.finalize()
        _CACHE["nc"] = nc
    nc = _CACHE["nc"]

    in_maps = []
    for i in range(NCORES):
        in_maps.append({
            "dist": np.ascontiguousarray(distance[i * BL : (i + 1) * BL]),
            "wpack": wpacks[i],
        })
    res = run_bass_kernel_spmd(nc, in_maps, list(range(NCORES)), trace=trace)
    out = np.concatenate([res.results[i]["out"] for i in range(NCORES)], axis=0)
    out = (out.astype(np.float64) + b_head).astype(np.float32)
    return out, res


def kernel(**inputs):
    out, _ = run(trace=False, **inputs)
    return out
